# revision 1
# baseline (speedup 1.0000x reference)
"""CapsuleLayer (dynamic routing) on 8 trn2 NeuronCores.

Math: u_hat[b,c,i,o] = sum_{d,k} W[c,0,i,o,d,k] x[b,i,k]
             = sum_k Wsum[c,i,o,k] x[b,i,k],  Wsum = W.sum(d)   (134MB -> 8.4MB)
Routing logits are cumulative: b_t = u_hat . (sum_{tau<t} v_tau), so each
iteration only needs the running vector-sum w.  Everything is sharded over
IN_CAPS (i) across 8 cores; only s[b,c,o] (131KB) crosses cores, reduced on
host between launches.

Launch 1 (per core, i-slice of 256):
  - reduce W over d -> Wsum, stored to HBM in two layouts:
      wa16 [c,(i k),o] (bf16, s-matmul lhsT) and wb2 [q,c,o,128] (f32, G lhsT)
  - transpose x -> xt [(i k), b] (bf16)
  - s0_partial[c,o,b] = sum_{ik in slice} Wsum^T x   (uniform-c iteration 0)
Launch 2/3 (same kernel, different w input):
  - G = Wsum^T_o w  (PE), P = x*G (DVE), b_t = sum_k P (PE E-fold)
  - softmax over classes -> c_t
  - crep = k-replicate(c_t) (PE), y = x*crep (DVE), s_part = Wsum^T y (PE)
Host: s = sum over cores, v = squash(s), w accumulates v.
"""

import contextlib

import numpy as np
import ml_dtypes  # noqa: F401  (bf16 array dtype for I/O maps)

import concourse.bass as bass
import concourse.mybir as mybir
import concourse.tile as tile
from concourse import masks
from concourse.bass_utils import run_bass_kernel_spmd
from bass_rust import ScopedClock

# ---------------------------------------------------------------- constants
C, I, O, D, K, B = 8, 2048, 16, 16, 8, 256
NCORES = 8
IL = I // NCORES          # 256 i's per core
CH = IL * K // 128        # 16 (i,k)-chunks of 128 partitions per core
F32 = mybir.dt.float32
F32R = mybir.dt.float32r
BF16 = mybir.dt.bfloat16
CB = C * B

# ------------------------------------------------- tile tail-drain workaround
_MAX_WAITS = 1


def _patched_drain_and_barrier(self, tick_clock, wait_clock):
    nc = self.nc
    drain_inst = nc.sync.drain()
    wait_clock.add_sem_waits(
        drain_inst.ins, ScopedClock({None: tick_clock.global_clock})
    )
    si = drain_inst.ins.sync_info
    if si is not None and si.on_wait and len(si.on_wait) > _MAX_WAITS:
        waits = list(si.on_wait)
        si.on_wait = waits[:_MAX_WAITS]
        for i in range(_MAX_WAITS, len(waits), _MAX_WAITS):
            extra = nc.sync.drain()
            extra.ins.sync_info = mybir.SyncInfo(
                on_wait=waits[i : i + _MAX_WAITS], on_update=[]
            )
    nc.all_engine_barrier()
    assert self.sems is not None
    popped = nc._tile_sem_poison_stack.pop()
    assert popped is self._sem_poison
    nc.clear_and_free_semaphores(list(self.sems.allocated().values()))
    nc.all_engine_barrier()


tile.TileContext._drain_and_barrier = _patched_drain_and_barrier

_fix_ctr = [0]


def fixup_multi_waits(nc):
    """walrus in this toolchain accepts at most one sem wait per instruction;
    hoist extra waits onto same-engine drains placed just before."""
    for f in nc.m.functions:
        for bb in f.blocks:
            out = []
            for inst in bb.instructions:
                si = inst.sync_info
                if si is not None and si.on_wait and len(si.on_wait) > _MAX_WAITS:
                    waits = list(si.on_wait)
                    for i in range(0, len(waits) - _MAX_WAITS, _MAX_WAITS):
                        _fix_ctr[0] += 1
                        d = mybir.InstDrain(
                            name=f"waitsplit_{_fix_ctr[0]}", ins=[], outs=[]
                        )
                        d.engine = inst.engine
                        d.sync_info = mybir.SyncInfo(
                            on_wait=waits[i : i + _MAX_WAITS], on_update=[]
                        )
                        out.append(d)
                    si.on_wait = waits[len(waits) - _MAX_WAITS :]
                out.append(inst)
            bb.instructions[:] = out
    return nc



def build_all(fixup=True):
    nc = bass.Bass("TRN2", target_bir_lowering=False, debug=False,
                   num_devices=NCORES)
    W_d = nc.dram_tensor("W", [C, IL, O, D, K], F32, kind="ExternalInput").ap()
    x_d = nc.dram_tensor("x", [B, IL, K], F32, kind="ExternalInput").ap()
    v_d = nc.dram_tensor("v", [C, O, B], F32R, kind="ExternalOutput").ap()
    wa16_d = nc.dram_tensor("wa16", [C, IL * K, O], BF16).ap()
    wb2_d = nc.dram_tensor("wb2", [CH, C, O, 128], BF16).ap()
    # collective bounce buffers (one pair per iteration)
    cc_in = [nc.dram_tensor(f"cc_in{t}", [16, CB], F32).ap() for t in range(3)]
    cc_out = [nc.dram_tensor(f"cc_out{t}", [16, CB], F32).ap() for t in range(3)]

    with tile.TileContext(nc) as tc:
        with (
            tc.tile_pool(name="const", bufs=1) as constp,
            tc.tile_pool(name="persist", bufs=1) as pers,
            tc.tile_pool(name="small", bufs=4) as smallp,
            tc.tile_pool(name="work", bufs=3) as workp,
            tc.tile_pool(name="soft", bufs=2) as softp,
            tc.tile_pool(name="sqpool", bufs=1) as sqp,
        ):
            # ---------------- constants
            ident = constp.tile([128, 128], F32)
            masks.make_identity(nc, ident[:])
            identb = constp.tile([128, 128], BF16)
            with nc.allow_low_precision(reason="identity copy"):
                nc.vector.tensor_copy(identb[:], ident[:])
            e_big = constp.tile([128, 256], BF16)
            nc.gpsimd.memset(e_big[:], 1.0)
            nc.gpsimd.affine_select(
                out=e_big[:], in_=e_big[:],
                compare_op=mybir.AluOpType.is_ge, fill=0.0,
                base=1024, pattern=[[-8, 256]], channel_multiplier=1)
            nc.gpsimd.affine_select(
                out=e_big[:], in_=e_big[:],
                compare_op=mybir.AluOpType.is_ge, fill=0.0,
                base=-1017, pattern=[[8, 256]], channel_multiplier=-1)
            e2_big = constp.tile([128, 1152], BF16)
            nc.gpsimd.memset(e2_big[:], 1.0)
            nc.gpsimd.affine_select(
                out=e2_big[:], in_=e2_big[:],
                compare_op=mybir.AluOpType.is_ge, fill=0.0,
                base=0, pattern=[[1, 1152]], channel_multiplier=-8)
            nc.gpsimd.affine_select(
                out=e2_big[:], in_=e2_big[:],
                compare_op=mybir.AluOpType.is_ge, fill=0.0,
                base=7, pattern=[[-1, 1152]], channel_multiplier=8)
            ones16f = constp.tile([16, 1], F32)
            nc.gpsimd.memset(ones16f[:], 1.0)
            ones16 = constp.tile([16, 1], F32R)
            ones1f = constp.tile([1, 16], F32)
            nc.gpsimd.memset(ones1f[:], 1.0)
            ones1 = constp.tile([1, 16], F32R)
            with nc.allow_low_precision(reason="ones copy"):
                nc.vector.tensor_copy(ones16[:], ones16f[:])
                nc.vector.tensor_copy(ones1[:], ones1f[:])

            # ---------------- persistent state
            xt16 = pers.tile([128, CH * B], BF16)
            wa_all = pers.tile([128, C * CH * O], BF16)
            w_acc = pers.tile([16, CB], BF16)
            bt_sb = pers.tile([128, 2 * CB], F32)
            ct_all = pers.tile([128, 2 * CB], BF16)

            # ---------------- phases A-C (scoped SBUF: xt32, W/x staging)
            phio_cm = contextlib.ExitStack()
            phio = phio_cm.enter_context(tc.tile_pool(name="phio", bufs=3))
            with tc.tile_pool(name="xps", bufs=4, space="PSUM") as xps:
                for bc in range(2):
                    xin = phio.tile([128, IL * K], F32, tag="xin", bufs=2)
                    nc.sync.dma_start(
                        xin[:],
                        x_d[bc * 128 : (bc + 1) * 128].rearrange("b i k -> b (i k)"),
                    )
                    for q in range(CH):
                        ps = xps.tile([128, 128], F32)
                        nc.tensor.transpose(
                            ps[:], xin[:, q * 128 : (q + 1) * 128], ident[:]
                        )
                        nc.scalar.copy(
                            xt16[:, q * B + bc * 128 : q * B + bc * 128 + 128],
                            ps[:],
                        )

            # ---------------- phase B: W reduce over d
            for t in range(2 * C):
                c, ih = t // 2, t % 2
                wt = phio.tile([128, O * D * K], F32, tag="wt", bufs=2)
                (nc.sync if t % 2 == 0 else nc.scalar).dma_start(
                    wt[:],
                    W_d[c, ih * 128 : (ih + 1) * 128].rearrange("p o d k -> p (o d k)"),
                )
                wf = smallp.tile([128, K * O], F32, tag="wf")
                nc.vector.reduce_sum(
                    wf[:].rearrange("p (k o) -> p o k", k=K),
                    wt[:].rearrange("p (o d k) -> p o k d", o=O, d=D, k=K),
                    axis=mybir.AxisListType.X,
                )
                wf16 = smallp.tile([128, K * O], BF16, tag="wf16")
                nc.vector.tensor_copy(wf16[:], wf[:])
                # flat contiguous write: dst row i <-> 128 els (k*16+o)
                dst16 = wa16_d[c].rearrange("(i f) o -> i (f o)", f=K)[
                    ih * 128 : (ih + 1) * 128
                ]
                nc.scalar.dma_start(dst16, wf16[:])

            # ---------------- phase C: round-trip -> s0 + wb2 + wa_all
            with (
                tc.tile_pool(name="tps", bufs=2, space="PSUM") as tpsp,
                tc.tile_pool(name="s0ps", bufs=2, space="PSUM") as s0ps,
            ):
                for c in range(C):
                    s0p = s0ps.tile([16, B], F32, tag="s0p")
                    tp = None
                    for q in range(CH):
                        wa = wa_all[:, c * CH * O + q * O : c * CH * O + (q + 1) * O]
                        (nc.sync if q % 2 else nc.scalar).dma_start(
                            wa, wa16_d[c, q * 128 : (q + 1) * 128]
                        )
                        nc.tensor.matmul(
                            s0p[:], wa, xt16[:, q * B : (q + 1) * B],
                            start=(q == 0), stop=(q == CH - 1),
                        )
                        if q % 4 == 0:
                            tp = tpsp.tile([16, 512], BF16, tag="tp",
                                           name=f"tp_{c}_{q}")
                        nc.tensor.transpose(
                            tp[:, (q % 4) * 128 : (q % 4) * 128 + 128],
                            wa, identb[:],
                        )
                        if q % 4 == 3:
                            wbp = phio.tile([16, 512], BF16, tag="wbp", bufs=2)
                            nc.scalar.copy(wbp[:], tp[:])
                            nc.scalar.dma_start(
                                wb2_d[q - 3 : q + 1, c].rearrange("q o s -> o q s"),
                                wbp[:].rearrange("o (q s) -> o q s", s=128),
                            )
                    s0sb = smallp.tile([16, B], F32, tag="s_sb", name=f"s0sb{c}")
                    nc.scalar.copy(s0sb[:], s0p[:])
                    nc.sync.dma_start(cc_in[0][:, c * B : (c + 1) * B], s0sb[:])

            # ---------------- allreduce + squash helper
            def allreduce_squash(t, pre, last):
                """cc_in[t] holds the local partial of s/pre; reduce, squash
                v = squash(pre * s_sum), accumulate into w_acc or emit v."""
                nc.gpsimd.collective_compute(
                    "AllReduce",
                    mybir.AluOpType.add,
                    replica_groups=[list(range(NCORES))],
                    ins=[cc_in[t].opt()],
                    outs=[cc_out[t].opt()],
                )
                s_sum = sqp.tile([16, CB], F32, tag="s_sum", name=f"s_sum{t}")
                nc.sync.dma_start(s_sum[:], cc_out[t][:, :])
                sq = sqp.tile([16, CB], F32R, tag="sq", name=f"sq{t}")
                nc.scalar.activation(
                    sq[:], s_sum[:], mybir.ActivationFunctionType.Square,
                    scale=pre,
                )
                with tc.tile_pool(name=f"sqps{t}", bufs=1, space="PSUM") as sqps:
                    ssq_ps = sqps.tile([1, CB], F32, tag="ssq")
                    for j in range(4):
                        nc.tensor.matmul(
                            ssq_ps[:, j * 512 : (j + 1) * 512],
                            ones16[:],
                            sq[:, j * 512 : (j + 1) * 512],
                            start=True, stop=True,
                        )
                    ssq_row = sqp.tile([1, CB], F32R, tag="row_tmp",
                                       name=f"ssq_row{t}")
                    nc.scalar.copy(ssq_row[:], ssq_ps[:])
                # reshape to [128, 16] for cheap elementwise math
                ssq = sqp.tile([128, 16], F32R, tag="ssq_rs", name=f"ssq_rs{t}")
                nc.sync.dma_start(
                    ssq[:], ssq_row[:].rearrange("u (p f) -> u p f", p=128)
                )
                den1 = sqp.tile([128, 16], F32, tag="den1", name=f"den1{t}")
                nc.vector.tensor_scalar_add(den1[:], ssq[:], 1.0)
                r1 = sqp.tile([128, 16], F32, tag="r1", name=f"r1{t}")
                nc.vector.reciprocal(r1[:], den1[:])
                rt = sqp.tile([128, 16], F32, tag="rt", name=f"rt{t}")
                nc.scalar.sqrt(rt[:], ssq[:])
                r2 = sqp.tile([128, 16], F32, tag="r2", name=f"r2{t}")
                nc.vector.reciprocal(r2[:], rt[:])
                m1 = sqp.tile([128, 16], F32, tag="m1", name=f"m1{t}")
                nc.vector.tensor_mul(m1[:], ssq[:], r1[:])
                scale_rs = sqp.tile([128, 16], F32R, tag="scale_rs",
                                    name=f"scale_rs{t}")
                nc.vector.tensor_mul(scale_rs[:], m1[:], r2[:])
                if pre != 1.0:
                    nc.vector.tensor_scalar_mul(scale_rs[:], scale_rs[:], pre)
                scale_row = sqp.tile([1, CB], F32R, tag="row_tmp",
                                     name=f"scale_row{t}")
                nc.sync.dma_start(
                    scale_row[:].rearrange("u (p f) -> u p f", p=128), scale_rs[:]
                )
                with tc.tile_pool(name=f"bcps{t}", bufs=1, space="PSUM") as bcps:
                    bc_ps = bcps.tile([16, CB], F32, tag="bc")
                    for j in range(4):
                        nc.tensor.matmul(
                            bc_ps[:, j * 512 : (j + 1) * 512],
                            ones1[:],
                            scale_row[:, j * 512 : (j + 1) * 512],
                            start=True, stop=True,
                        )
                    v_sb = sqp.tile([16, CB], F32R, tag="v_sbr",
                                    name=f"v_sbr{t}")
                    with nc.allow_low_precision(reason="f32r full range"):
                        nc.vector.tensor_mul(v_sb[:], s_sum[:], bc_ps[:])
                    if last:
                        for c in range(C):
                            nc.sync.dma_start(
                                v_d[c], v_sb[:, c * B : (c + 1) * B]
                            )
                    elif t == 0:
                        nc.vector.tensor_copy(w_acc[:], v_sb[:])
                    else:
                        with nc.allow_low_precision(reason="w accum"):
                            nc.vector.tensor_add(w_acc[:], w_acc[:], v_sb[:])

            phio_cm.close()

            allreduce_squash(0, 1.0 / C, last=False)

            # ---------------- routing iterations 1 and 2
            for it in range(1, 3):
                # phase 1: b_t
                with (
                    tc.tile_pool(name=f"btps{it}", bufs=1, space="PSUM") as btps,
                    tc.tile_pool(name=f"gps{it}", bufs=2, space="PSUM") as gps,
                ):
                    for h in range(2):
                        bt_ps = btps.tile([128, CB], F32, tag="bt")
                        for qq in range(CH // 2):
                            q = h * 8 + qq
                            wbq = workp.tile([16, C * 128], BF16, tag="wbq")
                            nc.sync.dma_start(
                                wbq[:].rearrange("o (c s) -> o c s", c=C),
                                wb2_d[q].rearrange("c o s -> o c s"),
                            )
                            p_sb = workp.tile([128, CB], BF16, tag="p_sb")
                            for piece in range(2):
                                g_ps = gps.tile([128, 1024], F32, tag="g")
                                for cc in range(4):
                                    c = piece * 4 + cc
                                    nc.tensor.matmul(
                                        g_ps[:, cc * B : (cc + 1) * B],
                                        wbq[:, c * 128 : (c + 1) * 128],
                                        w_acc[:, c * B : (c + 1) * B],
                                        start=True, stop=True,
                                    )
                                g_sb = workp.tile([128, 1024], BF16, tag="g_sb")
                                nc.scalar.copy(g_sb[:], g_ps[:])
                                xb = (
                                    xt16[:, q * B : (q + 1) * B]
                                    .rearrange("p (u b) -> p u b", u=1)
                                    .broadcast_to([128, 4, B])
                                )
                                nc.vector.tensor_mul(
                                    p_sb[:, piece * 1024 : (piece + 1) * 1024]
                                    .rearrange("p (c b) -> p c b", c=4),
                                    xb,
                                    g_sb[:].rearrange("p (c b) -> p c b", c=4),
                                )
                            eq = e_big[:, 128 - 16 * qq : 256 - 16 * qq]
                            for j in range(4):
                                nc.tensor.matmul(
                                    bt_ps[:, j * 512 : (j + 1) * 512],
                                    eq,
                                    p_sb[:, j * 512 : (j + 1) * 512],
                                    start=(qq == 0), stop=(qq == CH // 2 - 1),
                                )
                        nc.scalar.copy(bt_sb[:, h * CB : (h + 1) * CB], bt_ps[:])

                # phase 2: softmax over classes
                for h in range(2):
                    bt = bt_sb[:, h * CB : (h + 1) * CB]
                    rmax = softp.tile([128, B], F32, tag="rmax")
                    nc.vector.reduce_max(
                        rmax[:],
                        bt.rearrange("p (c b) -> p b c", c=C),
                        axis=mybir.AxisListType.X,
                    )
                    sub = sqp.tile([128, CB], F32, tag="sub")
                    nc.vector.tensor_sub(
                        sub[:].rearrange("p (c b) -> p c b", c=C),
                        bt.rearrange("p (c b) -> p c b", c=C),
                        rmax[:].rearrange("p (u b) -> p u b", u=1).broadcast_to(
                            [128, C, B]
                        ),
                    )
                    e_t = softp.tile([128, CB], BF16, tag="e_t")
                    nc.scalar.activation(
                        e_t[:], sub[:], mybir.ActivationFunctionType.Exp
                    )
                    den = softp.tile([128, B], F32, tag="den")
                    nc.vector.reduce_sum(
                        den[:],
                        e_t[:].rearrange("p (c b) -> p b c", c=C),
                        axis=mybir.AxisListType.X,
                    )
                    rec = softp.tile([128, B], F32, tag="rec")
                    nc.vector.reciprocal(rec[:], den[:])
                    nc.vector.tensor_mul(
                        ct_all[:, h * CB : (h + 1) * CB].rearrange(
                            "p (c b) -> p c b", c=C
                        ),
                        e_t[:].rearrange("p (c b) -> p c b", c=C),
                        rec[:].rearrange("p (u b) -> p u b", u=1).broadcast_to(
                            [128, C, B]
                        ),
                    )

                # phase 3+4: crep -> y -> s
                with (
                    tc.tile_pool(name=f"sps{it}", bufs=1, space="PSUM") as sps,
                    tc.tile_pool(name=f"crps{it}", bufs=2, space="PSUM") as crps,
                ):
                    for grp in range(2):
                        s_ps = [
                            sps.tile([16, B], F32, tag=f"s{cc}",
                                     name=f"s_ps{it}_{grp}_{cc}")
                            for cc in range(4)
                        ]
                        for q in range(CH):
                            h, qq = q // 8, q % 8
                            ct_half = ct_all[:, h * CB : (h + 1) * CB]
                            cr_ps = crps.tile([128, 1024], F32, tag="cr")
                            for j in range(2):
                                nc.tensor.matmul(
                                    cr_ps[:, j * 512 : (j + 1) * 512],
                                    e2_big[:, 128 * qq : 128 * qq + 128],
                                    ct_half[:, grp * 1024 + j * 512 :
                                            grp * 1024 + (j + 1) * 512],
                                    start=True, stop=True,
                                )
                            cr_sb = workp.tile([128, 1024], BF16, tag="cr_sb")
                            nc.scalar.copy(cr_sb[:], cr_ps[:])
                            xb = (
                                xt16[:, q * B : (q + 1) * B]
                                .rearrange("p (u b) -> p u b", u=1)
                                .broadcast_to([128, 4, B])
                            )
                            y_q = workp.tile([128, 1024], BF16, tag="y_q")
                            nc.vector.tensor_mul(
                                y_q[:].rearrange("p (c b) -> p c b", c=4),
                                xb,
                                cr_sb[:].rearrange("p (c b) -> p c b", c=4),
                            )
                            for cc in range(4):
                                c = grp * 4 + cc
                                nc.tensor.matmul(
                                    s_ps[cc][:],
                                    wa_all[:, c * CH * O + q * O :
                                           c * CH * O + (q + 1) * O],
                                    y_q[:, cc * B : (cc + 1) * B],
                                    start=(q == 0), stop=(q == CH - 1),
                                )
                        for cc in range(4):
                            c = grp * 4 + cc
                            s_sb = smallp.tile([16, B], F32, tag="s_sb",
                                               name=f"s_sb{it}_{c}")
                            nc.scalar.copy(s_sb[:], s_ps[cc][:])
                            nc.sync.dma_start(
                                cc_in[it][:, c * B : (c + 1) * B], s_sb[:]
                            )
                allreduce_squash(it, 1.0, last=(it == 2))
    return fixup_multi_waits(nc) if fixup else nc


_NC = None


def kernel(x: np.ndarray, W: np.ndarray, _timings=None) -> np.ndarray:
    global _NC
    x = np.asarray(x, np.float32)
    W = np.asarray(W, np.float32)
    if _NC is None:
        _NC = build_all()
    in_maps = []
    for j in range(NCORES):
        sl = slice(j * IL, (j + 1) * IL)
        in_maps.append(
            {
                "W": np.ascontiguousarray(W[:, 0, sl]),
                "x": np.ascontiguousarray(x[:, sl, :]),
            }
        )
    res = run_bass_kernel_spmd(
        _NC, in_maps, core_ids=list(range(NCORES)),
        trace=_timings is not None,
    )
    if _timings is not None:
        _timings.append(res.exec_time_ns)
    v = res.results[0]["v"].astype(np.float32)  # [C, O, B]
    return np.ascontiguousarray(v.transpose(2, 0, 1))



# revision 12
# speedup vs baseline: 1.3412x; 1.3412x over previous
"""CapsuleLayer (dynamic routing) on 8 trn2 NeuronCores — v2.

Math: u_hat[b,c,i,o] = sum_{d,k} W[c,0,i,o,d,k] x[b,i,k]
             = sum_k Wsum[c,i,o,k] x[b,i,k],  Wsum = W.sum(d)   (134MB -> 8.4MB)
Sharded over IN_CAPS (i) across 8 cores; only s[b,c,o] partials (131KB)
cross cores via on-device AllReduce.  Routing logits are cumulative:
b_t = u_hat . (sum_{tau<t} v_tau) so each iteration needs only the running
vector-sum w.

Layout (v2): partitions = i (2 halves of 128), free dims = (c,k,b) combos.
  xs[h]   [128,(k b)]     x transposed (PE), bf16
  wS[h]   [128,(c o k)]   Wsum, lhsT for s-matmuls (contract i)
  wGT[h]  [16,(c k i)]    Wsum^T, lhsT for G-matmuls (contract o)
Per iteration:
  G = Wsum^T w (PE, PSUM f32) ; P = x*G (DVE, psum-read) ; b_t = fold_k(P)
  (DVE halving adds — contiguous, no PE fold / no crep matmuls)
  softmax over c: exp (ACT) + halving adds over c + recip + mul (DVE)
  y_c = x * c_t (DVE broadcast over k) ; s = Wsum^T y (PE)
Squash after AllReduce: scale=sqrt(ssq)/(1+ssq) on [1,CB], broadcast via
ones-matmul (ones value carries the 1/C prefactor for t=0).
"""

import contextlib

import numpy as np
import ml_dtypes  # noqa: F401

import concourse.bass as bass
import concourse.mybir as mybir
import concourse.tile as tile
from concourse import masks
from concourse.bass_utils import run_bass_kernel_spmd
from bass_rust import ScopedClock

# ---------------------------------------------------------------- constants
C, I, O, D, K, B = 8, 2048, 16, 16, 8, 256
NCORES = 8
IL = I // NCORES          # 256 i's per core
NH = IL // 128            # 2 partition-halves
F32 = mybir.dt.float32
F32R = mybir.dt.float32r
BF16 = mybir.dt.bfloat16
CB = C * B
KB = K * B

# ------------------------------------------------- tile tail-drain workaround
_MAX_WAITS = 1


def _patched_drain_and_barrier(self, tick_clock, wait_clock):
    nc = self.nc
    drain_inst = nc.sync.drain()
    wait_clock.add_sem_waits(
        drain_inst.ins, ScopedClock({None: tick_clock.global_clock})
    )
    si = drain_inst.ins.sync_info
    if si is not None and si.on_wait and len(si.on_wait) > _MAX_WAITS:
        waits = list(si.on_wait)
        si.on_wait = waits[:_MAX_WAITS]
        for i in range(_MAX_WAITS, len(waits), _MAX_WAITS):
            extra = nc.sync.drain()
            extra.ins.sync_info = mybir.SyncInfo(
                on_wait=waits[i : i + _MAX_WAITS], on_update=[]
            )
    nc.all_engine_barrier()
    assert self.sems is not None
    popped = nc._tile_sem_poison_stack.pop()
    assert popped is self._sem_poison
    nc.clear_and_free_semaphores(list(self.sems.allocated().values()))
    nc.all_engine_barrier()


tile.TileContext._drain_and_barrier = _patched_drain_and_barrier

_fix_ctr = [0]


def fixup_multi_waits(nc):
    """walrus in this toolchain accepts at most one sem wait per instruction;
    hoist extra waits onto same-engine drains placed just before."""
    for f in nc.m.functions:
        for bb in f.blocks:
            out = []
            for inst in bb.instructions:
                si = inst.sync_info
                if si is not None and si.on_wait and len(si.on_wait) > _MAX_WAITS:
                    waits = list(si.on_wait)
                    for i in range(0, len(waits) - _MAX_WAITS, _MAX_WAITS):
                        _fix_ctr[0] += 1
                        d = mybir.InstDrain(
                            name=f"waitsplit_{_fix_ctr[0]}", ins=[], outs=[]
                        )
                        d.engine = inst.engine
                        d.sync_info = mybir.SyncInfo(
                            on_wait=waits[i : i + _MAX_WAITS], on_update=[]
                        )
                        out.append(d)
                    si.on_wait = waits[len(waits) - _MAX_WAITS :]
                out.append(inst)
            bb.instructions[:] = out
    return nc


def build_all(fixup=True, dbg=False):
    nc = bass.Bass("TRN2", target_bir_lowering=False, debug=False,
                   num_devices=NCORES)
    dbg_t = {}
    if dbg:
        for nm, shape, dt in [
            ("dbg_s0", [16, CB], F32), ("dbg_ssum", [16, CB], F32),
            ("dbg_wacc", [16, CB], BF16), ("dbg_ein", [128, CB], F32),
            ("dbg_ct", [128, CB], BF16), ("dbg_xs", [128, KB], BF16),
            ("dbg_wS", [128, C * 128], BF16),
            ("dbg_wGT", [16, C * K * 128], BF16),
        ]:
            dbg_t[nm] = nc.dram_tensor(nm, shape, dt,
                                       kind="ExternalOutput").ap()
    W_d = nc.dram_tensor("W", [C, IL, O, D, K], F32, kind="ExternalInput").ap()
    x_d = nc.dram_tensor("x", [B, IL, K], F32, kind="ExternalInput").ap()
    v_d = nc.dram_tensor("v", [C, O, B], F32R, kind="ExternalOutput").ap()
    cc_in = [nc.dram_tensor(f"cc_in{t}", [16, CB], F32).ap() for t in range(3)]
    cc_out = [nc.dram_tensor(f"cc_out{t}", [16, CB], F32).ap() for t in range(3)]

    with tile.TileContext(nc) as tc:
        with (
            tc.tile_pool(name="const", bufs=1) as constp,
            tc.tile_pool(name="persist", bufs=1) as pers,
            tc.tile_pool(name="small", bufs=4) as smallp,
            tc.tile_pool(name="work", bufs=3) as workp,
        ):
            # ---------------- constants
            ident = constp.tile([128, 128], F32)
            masks.make_identity(nc, ident[:])
            identb = constp.tile([128, 128], BF16)
            with nc.allow_low_precision(reason="identity copy"):
                nc.vector.tensor_copy(identb[:], ident[:])
            ones16f = constp.tile([16, 1], F32)
            nc.gpsimd.memset(ones16f[:], 1.0)
            ones16 = constp.tile([16, 1], BF16)
            onespre_f = constp.tile([1, 16], F32)
            nc.gpsimd.memset(onespre_f[:], 1.0 / C)
            onespre = constp.tile([1, 16], BF16)
            onesone_f = constp.tile([1, 16], F32)
            nc.gpsimd.memset(onesone_f[:], 1.0)
            onesone = constp.tile([1, 16], BF16)
            with nc.allow_low_precision(reason="ones copy"):
                nc.vector.tensor_copy(ones16[:], ones16f[:])
                nc.vector.tensor_copy(onespre[:], onespre_f[:])
                nc.vector.tensor_copy(onesone[:], onesone_f[:])

            # ---------------- persistent state
            xs = [pers.tile([128, KB], BF16, name=f"xs{h}") for h in range(NH)]
            wS = [pers.tile([128, C * 128], BF16, name=f"wS{h}")
                  for h in range(NH)]
            wGT = [pers.tile([16, C * K * 128], BF16, name=f"wGT{h}")
                   for h in range(NH)]
            w_acc = pers.tile([16, CB], BF16)
            e_in = [pers.tile([128, CB], F32, name=f"e_in{h}")
                    for h in range(NH)]
            ct = [pers.tile([128, CB], BF16, name=f"ct{h}") for h in range(NH)]

            # ---------------- phase A: x load + transpose to [i,(k b)]
            pha_cm = contextlib.ExitStack()
            phio = pha_cm.enter_context(tc.tile_pool(name="phio", bufs=4))
            bred = pha_cm.enter_context(tc.tile_pool(name="bred", bufs=2))
            with tc.tile_pool(name="xps", bufs=2, space="PSUM") as xps:
                for bc in range(2):
                    xin = phio.tile([128, IL * K], F32, tag="xin", bufs=2)
                    nc.sync.dma_start(
                        xin[:],
                        x_d[bc * 128 : (bc + 1) * 128].rearrange(
                            "b i k -> b (i k)"),
                    )
                    xv = xin[:].rearrange("b (i k) -> b i k", k=K)
                    for h in range(NH):
                        for k in range(K):
                            ps = xps.tile([128, 128], F32, tag="xt")
                            nc.tensor.transpose(
                                ps[:], xv[:, h * 128 : (h + 1) * 128, k],
                                ident[:],
                            )
                            eng = nc.scalar if (k % 2 == 0) else nc.vector
                            with nc.allow_low_precision(reason="x bf16"):
                                (eng.copy if eng is nc.scalar
                                 else eng.tensor_copy)(
                                    xs[h][:, k * B + bc * 128 :
                                          k * B + bc * 128 + 128],
                                    ps[:],
                                )

                # ---------------- phase B: W dma + d-reduce + layouts + s0
                dmae = [nc.sync, nc.scalar, nc.gpsimd]
                with (
                    tc.tile_pool(name="wgps", bufs=2, space="PSUM") as wgps,
                    tc.tile_pool(name="s0ps", bufs=2, space="PSUM") as s0ps,
                ):
                    s0_cur = None
                    for t in range(2 * C):
                        c, h = t // 2, t % 2
                        wt = phio.tile([128, O * D * K], F32, tag="wt", bufs=3)
                        dmae[t % 3].dma_start(
                            wt[:],
                            W_d[c, h * 128 : (h + 1) * 128].rearrange(
                                "p o d k -> p (o d k)"),
                        )
                        wv = wt[:].rearrange("p (o d k) -> p o d k", o=O, d=D,
                                             k=K)
                        w1 = bred.tile([128, O * 8 * K], F32, tag="w1")
                        w1v = w1[:].rearrange("p (o d k) -> p o d k", o=O, d=8,
                                              k=K)
                        nc.vector.tensor_add(w1v, wv[:, :, 0:8, :],
                                             wv[:, :, 8:16, :])
                        w2 = bred.tile([128, O * 4 * K], F32, tag="w2")
                        w2v = w2[:].rearrange("p (o d k) -> p o d k", o=O, d=4,
                                              k=K)
                        nc.vector.tensor_add(w2v, w1v[:, :, 0:4, :],
                                             w1v[:, :, 4:8, :])
                        w3 = bred.tile([128, O * 2 * K], F32, tag="w3")
                        w3v = w3[:].rearrange("p (o d k) -> p o d k", o=O, d=2,
                                              k=K)
                        nc.vector.tensor_add(w3v, w2v[:, :, 0:2, :],
                                             w2v[:, :, 2:4, :])
                        wfs = wS[h][:, c * 128 : (c + 1) * 128].rearrange(
                            "p (o k) -> p o k", o=O, k=K)
                        with nc.allow_low_precision(reason="Wsum bf16"):
                            nc.vector.tensor_add(
                                wfs, w3v[:, :, 0, :], w3v[:, :, 1, :])
                        # transposed layout for G-matmuls: [o=16, i] per (c,k)
                        wSv = wS[h][:].rearrange("p (c o k) -> p c o k", c=C,
                                                 o=O, k=K)
                        for k in range(K):
                            tp = wgps.tile([16, 128], BF16, tag="wgt")
                            nc.tensor.transpose(tp[:], wSv[:, c, :, k],
                                                identb[:])
                            eng = nc.scalar if (k % 2 == 0) else nc.vector
                            with nc.allow_low_precision(reason="WsumT bf16"):
                                (eng.copy if eng is nc.scalar
                                 else eng.tensor_copy)(
                                    wGT[h][:, (c * K + k) * 128 :
                                           (c * K + k + 1) * 128],
                                    tp[:],
                                )
                        # s0 partial: accumulate Wsum^T x over (h, k)
                        if h == 0:
                            s0_cur = s0ps.tile([16, B], F32, tag="s0",
                                               name=f"s0_{c}")
                        for k in range(K):
                            nc.tensor.matmul(
                                s0_cur[:],
                                wSv[:, c, :, k],
                                xs[h][:, k * B : (k + 1) * B],
                                start=(h == 0 and k == 0),
                                stop=(h == 1 and k == K - 1),
                            )
                        if h == 1:
                            s_sb = smallp.tile([16, B], F32, tag="s_sb",
                                               name=f"s0sb{c}")
                            nc.scalar.copy(s_sb[:], s0_cur[:])
                            nc.sync.dma_start(
                                cc_in[0][:, c * B : (c + 1) * B], s_sb[:])
                            if dbg:
                                nc.scalar.dma_start(
                                    dbg_t["dbg_s0"][:, c * B : (c + 1) * B],
                                    s_sb[:])

            if dbg:
                nc.sync.dma_start(dbg_t["dbg_xs"], xs[0][:])
                nc.sync.dma_start(dbg_t["dbg_wS"], wS[0][:])
                nc.sync.dma_start(dbg_t["dbg_wGT"], wGT[0][:])
            pha_cm.close()
            sq_cm = contextlib.ExitStack()
            sqp = sq_cm.enter_context(tc.tile_pool(name="sqpool", bufs=1))

            # ---------------- allreduce + squash helper
            def allreduce_squash(t, last):
                nc.gpsimd.collective_compute(
                    "AllReduce",
                    mybir.AluOpType.add,
                    replica_groups=[list(range(NCORES))],
                    ins=[cc_in[t].opt()],
                    outs=[cc_out[t].opt()],
                )
                ones_row = onespre if t == 0 else onesone
                pre = 1.0 / C if t == 0 else 1.0
                s_sum = sqp.tile([16, CB], F32, tag="s_sum", name=f"s_sum{t}")
                nc.sync.dma_start(s_sum[:], cc_out[t][:, :])
                if dbg and t == 0:
                    nc.scalar.dma_start(dbg_t["dbg_ssum"], s_sum[:])
                sq = sqp.tile([16, CB], BF16, tag="sq", name=f"sq{t}")
                nc.scalar.activation(
                    sq[:], s_sum[:], mybir.ActivationFunctionType.Square,
                    scale=pre,
                )
                with tc.tile_pool(name=f"sqps{t}", bufs=1,
                                  space="PSUM") as sqps:
                    ssq_ps = sqps.tile([1, CB], F32, tag="ssq")
                    for j in range(4):
                        nc.tensor.matmul(
                            ssq_ps[:, j * 512 : (j + 1) * 512],
                            ones16[:],
                            sq[:, j * 512 : (j + 1) * 512],
                            start=True, stop=True,
                        )
                    rt = sqp.tile([1, CB], BF16, tag="rt", name=f"rt{t}")
                    nc.scalar.sqrt(rt[:], ssq_ps[:])
                    den = sqp.tile([1, CB], BF16, tag="den", name=f"den{t}")
                    with nc.allow_low_precision(reason="den bf16"):
                        nc.vector.tensor_scalar_add(den[:], ssq_ps[:], 1.0)
                    rec = sqp.tile([1, CB], F32, tag="rec", name=f"rec{t}")
                    nc.vector.reciprocal(rec[:], den[:])
                    gg = sqp.tile([1, CB], BF16, tag="gg", name=f"gg{t}")
                    with nc.allow_low_precision(reason="gg bf16"):
                        nc.vector.tensor_mul(gg[:], rt[:], rec[:])
                with tc.tile_pool(name=f"bcps{t}", bufs=1,
                                  space="PSUM") as bcps:
                    bc_ps = bcps.tile([16, CB], F32, tag="bc")
                    for j in range(4):
                        nc.tensor.matmul(
                            bc_ps[:, j * 512 : (j + 1) * 512],
                            ones_row[:],
                            gg[:, j * 512 : (j + 1) * 512],
                            start=True, stop=True,
                        )
                    if last:
                        v_sb = sqp.tile([16, CB], F32R, tag="v_sb")
                        with nc.allow_low_precision(reason="f32r out"):
                            nc.vector.tensor_mul(v_sb[:], s_sum[:], bc_ps[:])
                        for c in range(C):
                            nc.sync.dma_start(
                                v_d[c], v_sb[:, c * B : (c + 1) * B])
                    elif t == 0:
                        with nc.allow_low_precision(reason="w bf16"):
                            nc.vector.tensor_mul(w_acc[:], s_sum[:], bc_ps[:])
                        if dbg:
                            nc.scalar.dma_start(dbg_t["dbg_wacc"], w_acc[:])
                    else:
                        vtmp = sqp.tile([16, CB], BF16, tag="vtmp")
                        with nc.allow_low_precision(reason="w bf16"):
                            nc.vector.tensor_mul(vtmp[:], s_sum[:], bc_ps[:])
                            nc.vector.tensor_add(w_acc[:], w_acc[:], vtmp[:])

            allreduce_squash(0, last=False)

            # ---------------- routing iterations 1 and 2
            for it in range(1, 3):
                # phase 1: G (PE) -> P = x*G (DVE) -> fold k (DVE) -> logits
                with tc.tile_pool(name=f"gps{it}", bufs=2,
                                  space="PSUM") as gps:
                    for h in range(NH):
                        for c in range(C):
                            g_ps = gps.tile([128, KB], F32, tag="g")
                            for k in range(K):
                                nc.tensor.matmul(
                                    g_ps[:, k * B : (k + 1) * B],
                                    wGT[h][:, (c * K + k) * 128 :
                                           (c * K + k + 1) * 128],
                                    w_acc[:, c * B : (c + 1) * B],
                                    start=True, stop=True,
                                )
                            pm = workp.tile([128, KB], BF16, tag="pm", bufs=2)
                            with nc.allow_low_precision(reason="P bf16"):
                                nc.vector.tensor_mul(pm[:], xs[h][:], g_ps[:])
                            f1 = workp.tile([128, KB // 2], BF16, tag="f1")
                            with nc.allow_low_precision(reason="fold bf16"):
                                nc.vector.tensor_add(
                                    f1[:], pm[:, 0 : KB // 2],
                                    pm[:, KB // 2 : KB])
                                f2 = workp.tile([128, KB // 4], BF16,
                                                tag="f2")
                                nc.vector.tensor_add(
                                    f2[:], f1[:, 0 : KB // 4],
                                    f1[:, KB // 4 : KB // 2])
                            nc.vector.tensor_add(
                                e_in[h][:, c * B : (c + 1) * B],
                                f2[:, 0:B], f2[:, B : 2 * B])
                        # phase 2: softmax over classes (per half)
                        m1 = workp.tile([128, CB // 2], F32, tag="m1")
                        nc.vector.tensor_max(m1[:], e_in[h][:, 0 : CB // 2],
                                             e_in[h][:, CB // 2 : CB])
                        m2 = workp.tile([128, CB // 4], F32, tag="m2")
                        nc.vector.tensor_max(m2[:], m1[:, 0 : CB // 4],
                                             m1[:, CB // 4 : CB // 2])
                        rmax = workp.tile([128, B], F32, tag="rmax")
                        nc.vector.tensor_max(rmax[:], m2[:, 0:B],
                                             m2[:, B : 2 * B])
                        nc.vector.tensor_sub(
                            e_in[h][:].rearrange("p (c b) -> p c b", c=C),
                            e_in[h][:].rearrange("p (c b) -> p c b", c=C),
                            rmax[:].rearrange("p (u b) -> p u b", u=1)
                            .broadcast_to([128, C, B]),
                        )
                        e_t = workp.tile([128, CB], BF16, tag="e_t", bufs=2)
                        nc.scalar.activation(
                            e_t[:], e_in[h][:],
                            mybir.ActivationFunctionType.Exp)
                        d1 = workp.tile([128, CB // 2], BF16, tag="d1")
                        d2 = workp.tile([128, CB // 4], BF16, tag="d2")
                        den = workp.tile([128, B], F32, tag="sden")
                        with nc.allow_low_precision(reason="den bf16"):
                            nc.vector.tensor_add(
                                d1[:], e_t[:, 0 : CB // 2],
                                e_t[:, CB // 2 : CB])
                            nc.vector.tensor_add(
                                d2[:], d1[:, 0 : CB // 4],
                                d1[:, CB // 4 : CB // 2])
                        nc.vector.tensor_add(den[:], d2[:, 0:B],
                                             d2[:, B : 2 * B])
                        rec = workp.tile([128, B], F32, tag="srec")
                        nc.vector.reciprocal(rec[:], den[:])
                        rec16 = workp.tile([128, B], BF16, tag="srec16")
                        with nc.allow_low_precision(reason="rec bf16"):
                            nc.scalar.copy(rec16[:], rec[:])
                            nc.vector.tensor_mul(
                                ct[h][:].rearrange("p (c b) -> p c b", c=C),
                                e_t[:].rearrange("p (c b) -> p c b", c=C),
                                rec16[:].rearrange("p (u b) -> p u b", u=1)
                                .broadcast_to([128, C, B]),
                            )

                if dbg and it == 1:
                    nc.sync.dma_start(dbg_t["dbg_ein"], e_in[0][:])
                    nc.sync.dma_start(dbg_t["dbg_ct"], ct[0][:])
                # phase 3: y = x*c (DVE) ; s = Wsum^T y (PE)
                with tc.tile_pool(name=f"sps{it}", bufs=2,
                                  space="PSUM") as sps:
                    for c in range(C):
                        s_ps = sps.tile([16, B], F32, tag="s",
                                        name=f"s_ps{it}_{c}")
                        for h in range(NH):
                            y = workp.tile([128, KB], BF16, tag="y", bufs=2)
                            with nc.allow_low_precision(reason="y bf16"):
                                nc.vector.tensor_mul(
                                    y[:].rearrange("p (k b) -> p k b", k=K),
                                    xs[h][:].rearrange("p (k b) -> p k b",
                                                       k=K),
                                    ct[h][:, c * B : (c + 1) * B]
                                    .rearrange("p (u b) -> p u b", u=1)
                                    .broadcast_to([128, K, B]),
                                )
                            wSv = wS[h][:].rearrange("p (c o k) -> p c o k",
                                                     c=C, o=O, k=K)
                            for k in range(K):
                                nc.tensor.matmul(
                                    s_ps[:],
                                    wSv[:, c, :, k],
                                    y[:, k * B : (k + 1) * B],
                                    start=(h == 0 and k == 0),
                                    stop=(h == 1 and k == K - 1),
                                )
                        s_sb = smallp.tile([16, B], F32, tag="s_sb",
                                           name=f"s_sb{it}_{c}")
                        nc.scalar.copy(s_sb[:], s_ps[:])
                        nc.sync.dma_start(
                            cc_in[it][:, c * B : (c + 1) * B], s_sb[:])
                allreduce_squash(it, last=(it == 2))
            sq_cm.close()
    return fixup_multi_waits(nc) if fixup else nc


_NC = None


def kernel(x: np.ndarray, W: np.ndarray, _timings=None) -> np.ndarray:
    global _NC
    x = np.asarray(x, np.float32)
    W = np.asarray(W, np.float32)
    if _NC is None:
        _NC = build_all()
    in_maps = []
    for j in range(NCORES):
        sl = slice(j * IL, (j + 1) * IL)
        in_maps.append(
            {
                "W": np.ascontiguousarray(W[:, 0, sl]),
                "x": np.ascontiguousarray(x[:, sl, :]),
            }
        )
    res = run_bass_kernel_spmd(
        _NC, in_maps, core_ids=list(range(NCORES)),
        trace=_timings is not None,
    )
    if _timings is not None:
        _timings.append(res.exec_time_ns)
    v = res.results[0]["v"].astype(np.float32)  # [C, O, B]
    return np.ascontiguousarray(v.transpose(2, 0, 1))


# revision 16
# speedup vs baseline: 1.7802x; 1.3273x over previous
"""CapsuleLayer (dynamic routing) on 8 trn2 NeuronCores — v2.

Math: u_hat[b,c,i,o] = sum_{d,k} W[c,0,i,o,d,k] x[b,i,k]
             = sum_k Wsum[c,i,o,k] x[b,i,k],  Wsum = W.sum(d)   (134MB -> 8.4MB)
Sharded over IN_CAPS (i) across 8 cores; only s[b,c,o] partials (131KB)
cross cores via on-device AllReduce.  Routing logits are cumulative:
b_t = u_hat . (sum_{tau<t} v_tau) so each iteration needs only the running
vector-sum w.

Layout (v2): partitions = i (2 halves of 128), free dims = (c,k,b) combos.
  xs[h]   [128,(k b)]     x transposed (PE), bf16
  wS[h]   [128,(c o k)]   Wsum, lhsT for s-matmuls (contract i)
  wGT[h]  [16,(c k i)]    Wsum^T, lhsT for G-matmuls (contract o)
Per iteration:
  G = Wsum^T w (PE, PSUM f32) ; P = x*G (DVE, psum-read) ; b_t = fold_k(P)
  (DVE halving adds — contiguous, no PE fold / no crep matmuls)
  softmax over c: exp (ACT) + halving adds over c + recip + mul (DVE)
  y_c = x * c_t (DVE broadcast over k) ; s = Wsum^T y (PE)
Squash after AllReduce: scale=sqrt(ssq)/(1+ssq) on [1,CB], broadcast via
ones-matmul (ones value carries the 1/C prefactor for t=0).
"""

import contextlib

import numpy as np
import ml_dtypes  # noqa: F401

import concourse.bass as bass
import concourse.mybir as mybir
import concourse.tile as tile
from concourse import masks
from concourse.bass_utils import run_bass_kernel_spmd
from bass_rust import ScopedClock

# ---------------------------------------------------------------- constants
C, I, O, D, K, B = 8, 2048, 16, 16, 8, 256
NCORES = 8
IL = I // NCORES          # 256 i's per core
NH = IL // 128            # 2 partition-halves
F32 = mybir.dt.float32
F32R = mybir.dt.float32r
BF16 = mybir.dt.bfloat16
CB = C * B
KB = K * B

# ------------------------------------------------- tile tail-drain workaround
_MAX_WAITS = 1


def _patched_drain_and_barrier(self, tick_clock, wait_clock):
    nc = self.nc
    drain_inst = nc.sync.drain()
    wait_clock.add_sem_waits(
        drain_inst.ins, ScopedClock({None: tick_clock.global_clock})
    )
    si = drain_inst.ins.sync_info
    if si is not None and si.on_wait and len(si.on_wait) > _MAX_WAITS:
        waits = list(si.on_wait)
        si.on_wait = waits[:_MAX_WAITS]
        for i in range(_MAX_WAITS, len(waits), _MAX_WAITS):
            extra = nc.sync.drain()
            extra.ins.sync_info = mybir.SyncInfo(
                on_wait=waits[i : i + _MAX_WAITS], on_update=[]
            )
    nc.all_engine_barrier()
    assert self.sems is not None
    popped = nc._tile_sem_poison_stack.pop()
    assert popped is self._sem_poison
    nc.clear_and_free_semaphores(list(self.sems.allocated().values()))
    nc.all_engine_barrier()


tile.TileContext._drain_and_barrier = _patched_drain_and_barrier

_fix_ctr = [0]


def fixup_multi_waits(nc):
    """walrus in this toolchain accepts at most one sem wait per instruction;
    hoist extra waits onto same-engine drains placed just before."""
    for f in nc.m.functions:
        for bb in f.blocks:
            out = []
            for inst in bb.instructions:
                si = inst.sync_info
                if si is not None and si.on_wait and len(si.on_wait) > _MAX_WAITS:
                    waits = list(si.on_wait)
                    for i in range(0, len(waits) - _MAX_WAITS, _MAX_WAITS):
                        _fix_ctr[0] += 1
                        d = mybir.InstDrain(
                            name=f"waitsplit_{_fix_ctr[0]}", ins=[], outs=[]
                        )
                        d.engine = inst.engine
                        d.sync_info = mybir.SyncInfo(
                            on_wait=waits[i : i + _MAX_WAITS], on_update=[]
                        )
                        out.append(d)
                    si.on_wait = waits[len(waits) - _MAX_WAITS :]
                out.append(inst)
            bb.instructions[:] = out
    return nc


def build_all(fixup=True, dbg=False):
    nc = bass.Bass("TRN2", target_bir_lowering=False, debug=False,
                   num_devices=NCORES)
    dbg_t = {}
    if dbg:
        for nm, shape, dt in [
            ("dbg_s0", [16, CB], F32), ("dbg_ssum", [16, CB], F32),
            ("dbg_wacc", [16, CB], BF16), ("dbg_ein", [128, CB], F32),
            ("dbg_ct", [128, CB], BF16), ("dbg_xs", [128, KB], BF16),
            ("dbg_wS", [128, C * 128], BF16),
            ("dbg_wGT", [16, C * K * 128], BF16),
        ]:
            dbg_t[nm] = nc.dram_tensor(nm, shape, dt,
                                       kind="ExternalOutput").ap()
    W_d = nc.dram_tensor("W", [C, IL, O, D, K], F32, kind="ExternalInput").ap()
    x_d = nc.dram_tensor("x", [B, IL, K], F32, kind="ExternalInput").ap()
    v_d = nc.dram_tensor("v", [C, O, B], F32R, kind="ExternalOutput").ap()
    cc_in = [nc.dram_tensor(f"cc_in{t}", [16, CB], F32).ap() for t in range(3)]
    cc_out = [nc.dram_tensor(f"cc_out{t}", [16, CB], F32).ap() for t in range(3)]

    with tile.TileContext(nc) as tc:
        with (
            tc.tile_pool(name="const", bufs=1) as constp,
            tc.tile_pool(name="persist", bufs=1) as pers,
            tc.tile_pool(name="small", bufs=4) as smallp,
            tc.tile_pool(name="work", bufs=3) as workp,
        ):
            # ---------------- constants
            ident = constp.tile([128, 128], F32)
            masks.make_identity(nc, ident[:])
            identb = constp.tile([128, 128], BF16)
            with nc.allow_low_precision(reason="identity copy"):
                nc.vector.tensor_copy(identb[:], ident[:])
            ones16f = constp.tile([16, 1], F32)
            nc.gpsimd.memset(ones16f[:], 1.0)
            ones16 = constp.tile([16, 1], BF16)
            onespre_f = constp.tile([1, 16], F32)
            nc.gpsimd.memset(onespre_f[:], 1.0 / C)
            onespre = constp.tile([1, 16], BF16)
            onesone_f = constp.tile([1, 16], F32)
            nc.gpsimd.memset(onesone_f[:], 1.0)
            onesone = constp.tile([1, 16], BF16)
            with nc.allow_low_precision(reason="ones copy"):
                nc.vector.tensor_copy(ones16[:], ones16f[:])
                nc.vector.tensor_copy(onespre[:], onespre_f[:])
                nc.vector.tensor_copy(onesone[:], onesone_f[:])

            # ---------------- persistent state
            xs = [pers.tile([128, KB], BF16, name=f"xs{h}") for h in range(NH)]
            wS = [pers.tile([128, C * 128], BF16, name=f"wS{h}")
                  for h in range(NH)]
            wGT = [pers.tile([16, C * K * 128], BF16, name=f"wGT{h}")
                   for h in range(NH)]
            w_acc = pers.tile([16, CB], BF16)
            e_in = [pers.tile([128, CB], F32, name=f"e_in{h}")
                    for h in range(NH)]
            ct = [pers.tile([128, CB], BF16, name=f"ct{h}") for h in range(NH)]

            # ---------------- phase A: x load + transpose to [i,(k b)]
            pha_cm = contextlib.ExitStack()
            phio = pha_cm.enter_context(tc.tile_pool(name="phio", bufs=4))
            bred = pha_cm.enter_context(tc.tile_pool(name="bred", bufs=2))
            with tc.tile_pool(name="xps", bufs=2, space="PSUM") as xps:
                for bc in range(2):
                    xin = phio.tile([128, IL * K], F32, tag="xin", bufs=2)
                    nc.sync.dma_start(
                        xin[:],
                        x_d[bc * 128 : (bc + 1) * 128].rearrange(
                            "b i k -> b (i k)"),
                    )
                    xv = xin[:].rearrange("b (i k) -> b i k", k=K)
                    for h in range(NH):
                        for k in range(K):
                            ps = xps.tile([128, 128], F32, tag="xt")
                            nc.tensor.transpose(
                                ps[:], xv[:, h * 128 : (h + 1) * 128, k],
                                ident[:],
                            )
                            eng = nc.scalar if (k % 2 == 0) else nc.vector
                            with nc.allow_low_precision(reason="x bf16"):
                                (eng.copy if eng is nc.scalar
                                 else eng.tensor_copy)(
                                    xs[h][:, k * B + bc * 128 :
                                          k * B + bc * 128 + 128],
                                    ps[:],
                                )

                # ---------------- phase B: W dma + d-reduce + layouts + s0
                dmae = [nc.sync, nc.scalar, nc.gpsimd]
                with (
                    tc.tile_pool(name="wgps", bufs=2, space="PSUM") as wgps,
                    tc.tile_pool(name="s0ps", bufs=2, space="PSUM") as s0ps,
                ):
                    s0_cur = None
                    for t in range(2 * C):
                        c, h = t // 2, t % 2
                        wt = phio.tile([128, O * D * K], F32, tag="wt", bufs=3)
                        dmae[t % 3].dma_start(
                            wt[:],
                            W_d[c, h * 128 : (h + 1) * 128].rearrange(
                                "p o d k -> p (o d k)"),
                        )
                        wv = wt[:].rearrange("p (o d k) -> p o d k", o=O, d=D,
                                             k=K)
                        w1 = bred.tile([128, O * 8 * K], F32, tag="w1")
                        w1v = w1[:].rearrange("p (o d k) -> p o d k", o=O, d=8,
                                              k=K)
                        nc.vector.tensor_add(w1v, wv[:, :, 0:8, :],
                                             wv[:, :, 8:16, :])
                        w2 = bred.tile([128, O * 4 * K], F32, tag="w2")
                        w2v = w2[:].rearrange("p (o d k) -> p o d k", o=O, d=4,
                                              k=K)
                        nc.vector.tensor_add(w2v, w1v[:, :, 0:4, :],
                                             w1v[:, :, 4:8, :])
                        w3 = bred.tile([128, O * 2 * K], F32, tag="w3")
                        w3v = w3[:].rearrange("p (o d k) -> p o d k", o=O, d=2,
                                              k=K)
                        nc.vector.tensor_add(w3v, w2v[:, :, 0:2, :],
                                             w2v[:, :, 2:4, :])
                        wfs = wS[h][:, c * 128 : (c + 1) * 128].rearrange(
                            "p (o k) -> p o k", o=O, k=K)
                        with nc.allow_low_precision(reason="Wsum bf16"):
                            nc.vector.tensor_add(
                                wfs, w3v[:, :, 0, :], w3v[:, :, 1, :])
                        # transposed layout for G-matmuls: [o=16, i] per (c,k)
                        wSv = wS[h][:].rearrange("p (c o k) -> p c o k", c=C,
                                                 o=O, k=K)
                        for k in range(K):
                            tp = wgps.tile([16, 128], BF16, tag="wgt")
                            nc.tensor.transpose(tp[:], wSv[:, c, :, k],
                                                identb[:])
                            eng = nc.scalar if (k % 2 == 0) else nc.vector
                            with nc.allow_low_precision(reason="WsumT bf16"):
                                (eng.copy if eng is nc.scalar
                                 else eng.tensor_copy)(
                                    wGT[h][:, (c * K + k) * 128 :
                                           (c * K + k + 1) * 128],
                                    tp[:],
                                )
                        # s0 partial: accumulate Wsum^T x over (h, k)
                        if h == 0:
                            s0_cur = s0ps.tile([16, B], F32, tag="s0",
                                               name=f"s0_{c}")
                        for k in range(K):
                            nc.tensor.matmul(
                                s0_cur[:],
                                wSv[:, c, :, k],
                                xs[h][:, k * B : (k + 1) * B],
                                start=(h == 0 and k == 0),
                                stop=(h == 1 and k == K - 1),
                            )
                        if h == 1:
                            s_sb = smallp.tile([16, B], F32, tag="s_sb",
                                               name=f"s0sb{c}")
                            nc.scalar.copy(s_sb[:], s0_cur[:])
                            nc.sync.dma_start(
                                cc_in[0][:, c * B : (c + 1) * B], s_sb[:])
                            if dbg:
                                nc.scalar.dma_start(
                                    dbg_t["dbg_s0"][:, c * B : (c + 1) * B],
                                    s_sb[:])

            if dbg:
                nc.sync.dma_start(dbg_t["dbg_xs"], xs[0][:])
                nc.sync.dma_start(dbg_t["dbg_wS"], wS[0][:])
                nc.sync.dma_start(dbg_t["dbg_wGT"], wGT[0][:])
            pha_cm.close()
            sq_cm = contextlib.ExitStack()
            sqp = sq_cm.enter_context(tc.tile_pool(name="sqpool", bufs=1))

            # ---------------- allreduce + squash helper
            def allreduce_squash(t, last):
                nc.gpsimd.collective_compute(
                    "AllReduce",
                    mybir.AluOpType.add,
                    replica_groups=[list(range(NCORES))],
                    ins=[cc_in[t].opt()],
                    outs=[cc_out[t].opt()],
                )
                ones_row = onespre if t == 0 else onesone
                pre = 1.0 / C if t == 0 else 1.0
                s_sum = sqp.tile([16, CB], F32, tag="s_sum", name=f"s_sum{t}")
                nc.sync.dma_start(s_sum[:], cc_out[t][:, :])
                if dbg and t == 0:
                    nc.scalar.dma_start(dbg_t["dbg_ssum"], s_sum[:])
                sq = sqp.tile([16, CB], BF16, tag="sq", name=f"sq{t}")
                nc.scalar.activation(
                    sq[:], s_sum[:], mybir.ActivationFunctionType.Square,
                    scale=pre,
                )
                with tc.tile_pool(name=f"sqps{t}", bufs=1,
                                  space="PSUM") as sqps:
                    ssq_ps = sqps.tile([1, CB], F32, tag="ssq")
                    for j in range(4):
                        nc.tensor.matmul(
                            ssq_ps[:, j * 512 : (j + 1) * 512],
                            ones16[:],
                            sq[:, j * 512 : (j + 1) * 512],
                            start=True, stop=True,
                        )
                    ssq_row = sqp.tile([1, CB], F32, tag="ssq_row",
                                       name=f"ssq_row{t}")
                    nc.scalar.copy(ssq_row[:], ssq_ps[:])
                    ssq128 = sqp.tile([128, 16], F32, tag="ssq128",
                                      name=f"ssq128_{t}")
                    nc.sync.dma_start(
                        ssq128[:],
                        ssq_row[:].rearrange("u (p f) -> u p f", p=128))
                    rt = sqp.tile([128, 16], F32, tag="rt", name=f"rt{t}")
                    nc.scalar.sqrt(rt[:], ssq128[:])
                    den = sqp.tile([128, 16], F32, tag="den", name=f"den{t}")
                    nc.vector.tensor_scalar_add(den[:], ssq128[:], 1.0)
                    rec = sqp.tile([128, 16], F32, tag="rec", name=f"rec{t}")
                    nc.vector.reciprocal(rec[:], den[:])
                    gg128 = sqp.tile([128, 16], BF16, tag="gg128",
                                     name=f"gg128_{t}")
                    with nc.allow_low_precision(reason="gg bf16"):
                        nc.vector.tensor_mul(gg128[:], rt[:], rec[:])
                    gg = sqp.tile([1, CB], BF16, tag="gg", name=f"gg{t}")
                    nc.scalar.dma_start(
                        gg[:].rearrange("u (p f) -> u p f", p=128), gg128[:])
                with tc.tile_pool(name=f"bcps{t}", bufs=1,
                                  space="PSUM") as bcps:
                    bc_ps = bcps.tile([16, CB], F32, tag="bc")
                    for j in range(4):
                        nc.tensor.matmul(
                            bc_ps[:, j * 512 : (j + 1) * 512],
                            ones_row[:],
                            gg[:, j * 512 : (j + 1) * 512],
                            start=True, stop=True,
                        )
                    if last:
                        v_sb = sqp.tile([16, CB], F32R, tag="v_sb")
                        with nc.allow_low_precision(reason="f32r out"):
                            nc.vector.tensor_mul(v_sb[:], s_sum[:], bc_ps[:])
                        for c in range(C):
                            nc.sync.dma_start(
                                v_d[c], v_sb[:, c * B : (c + 1) * B])
                    elif t == 0:
                        with nc.allow_low_precision(reason="w bf16"):
                            nc.vector.tensor_mul(w_acc[:], s_sum[:], bc_ps[:])
                        if dbg:
                            nc.scalar.dma_start(dbg_t["dbg_wacc"], w_acc[:])
                    else:
                        vtmp = sqp.tile([16, CB], BF16, tag="vtmp")
                        with nc.allow_low_precision(reason="w bf16"):
                            nc.vector.tensor_mul(vtmp[:], s_sum[:], bc_ps[:])
                            nc.vector.tensor_add(w_acc[:], w_acc[:], vtmp[:])

            allreduce_squash(0, last=False)

            # ---------------- routing iterations 1 and 2
            for it in range(1, 3):
                # phase 1: G (PE) -> P = x*G (DVE) -> fold k (DVE) -> logits
                with tc.tile_pool(name=f"gps{it}", bufs=2,
                                  space="PSUM") as gps:
                    for h in range(NH):
                        for c in range(C):
                            g_ps = gps.tile([128, KB], F32, tag="g")
                            for k in range(K):
                                nc.tensor.matmul(
                                    g_ps[:, k * B : (k + 1) * B],
                                    wGT[h][:, (c * K + k) * 128 :
                                           (c * K + k + 1) * 128],
                                    w_acc[:, c * B : (c + 1) * B],
                                    start=True, stop=True,
                                )
                            g16 = workp.tile([128, KB], BF16, tag="g16",
                                             bufs=2)
                            nc.scalar.copy(g16[:], g_ps[:])
                            pm = workp.tile([128, KB], BF16, tag="pm", bufs=2)
                            with nc.allow_low_precision(reason="P bf16"):
                                nc.vector.tensor_mul(pm[:], xs[h][:], g16[:])
                            f1 = workp.tile([128, KB // 2], BF16, tag="f1", bufs=2)
                            with nc.allow_low_precision(reason="fold bf16"):
                                nc.vector.tensor_add(
                                    f1[:], pm[:, 0 : KB // 2],
                                    pm[:, KB // 2 : KB])
                                f2 = workp.tile([128, KB // 4], BF16,
                                                tag="f2", bufs=2)
                                nc.vector.tensor_add(
                                    f2[:], f1[:, 0 : KB // 4],
                                    f1[:, KB // 4 : KB // 2])
                            nc.vector.tensor_add(
                                e_in[h][:, c * B : (c + 1) * B],
                                f2[:, 0:B], f2[:, B : 2 * B])
                        # phase 2: softmax over classes (per half)
                        m1 = workp.tile([128, CB // 2], F32, tag="m1", bufs=1)
                        nc.vector.tensor_max(m1[:], e_in[h][:, 0 : CB // 2],
                                             e_in[h][:, CB // 2 : CB])
                        m2 = workp.tile([128, CB // 4], F32, tag="m2", bufs=1)
                        nc.vector.tensor_max(m2[:], m1[:, 0 : CB // 4],
                                             m1[:, CB // 4 : CB // 2])
                        rmax = workp.tile([128, B], F32, tag="rmax", bufs=2)
                        nc.vector.tensor_max(rmax[:], m2[:, 0:B],
                                             m2[:, B : 2 * B])
                        nc.vector.tensor_sub(
                            e_in[h][:].rearrange("p (c b) -> p c b", c=C),
                            e_in[h][:].rearrange("p (c b) -> p c b", c=C),
                            rmax[:].rearrange("p (u b) -> p u b", u=1)
                            .broadcast_to([128, C, B]),
                        )
                        e_t = workp.tile([128, CB], BF16, tag="e_t", bufs=2)
                        nc.scalar.activation(
                            e_t[:], e_in[h][:],
                            mybir.ActivationFunctionType.Exp)
                        d1 = workp.tile([128, CB // 2], BF16, tag="d1", bufs=1)
                        d2 = workp.tile([128, CB // 4], BF16, tag="d2", bufs=1)
                        den = workp.tile([128, B], F32, tag="sden", bufs=2)
                        with nc.allow_low_precision(reason="den bf16"):
                            nc.vector.tensor_add(
                                d1[:], e_t[:, 0 : CB // 2],
                                e_t[:, CB // 2 : CB])
                            nc.vector.tensor_add(
                                d2[:], d1[:, 0 : CB // 4],
                                d1[:, CB // 4 : CB // 2])
                        nc.vector.tensor_add(den[:], d2[:, 0:B],
                                             d2[:, B : 2 * B])
                        rec = workp.tile([128, B], F32, tag="srec", bufs=2)
                        nc.vector.reciprocal(rec[:], den[:])
                        rec16 = workp.tile([128, B], BF16, tag="srec16", bufs=2)
                        with nc.allow_low_precision(reason="rec bf16"):
                            nc.scalar.copy(rec16[:], rec[:])
                            nc.vector.tensor_mul(
                                ct[h][:].rearrange("p (c b) -> p c b", c=C),
                                e_t[:].rearrange("p (c b) -> p c b", c=C),
                                rec16[:].rearrange("p (u b) -> p u b", u=1)
                                .broadcast_to([128, C, B]),
                            )

                if dbg and it == 1:
                    nc.sync.dma_start(dbg_t["dbg_ein"], e_in[0][:])
                    nc.sync.dma_start(dbg_t["dbg_ct"], ct[0][:])
                # phase 3: y = x*c (DVE) ; s = Wsum^T y (PE)
                with tc.tile_pool(name=f"sps{it}", bufs=2,
                                  space="PSUM") as sps:
                    for c in range(C):
                        s_ps = sps.tile([16, B], F32, tag="s",
                                        name=f"s_ps{it}_{c}")
                        for h in range(NH):
                            y = workp.tile([128, KB], BF16, tag="y", bufs=2)
                            with nc.allow_low_precision(reason="y bf16"):
                                nc.vector.tensor_mul(
                                    y[:].rearrange("p (k b) -> p k b", k=K),
                                    xs[h][:].rearrange("p (k b) -> p k b",
                                                       k=K),
                                    ct[h][:, c * B : (c + 1) * B]
                                    .rearrange("p (u b) -> p u b", u=1)
                                    .broadcast_to([128, K, B]),
                                )
                            wSv = wS[h][:].rearrange("p (c o k) -> p c o k",
                                                     c=C, o=O, k=K)
                            for k in range(K):
                                nc.tensor.matmul(
                                    s_ps[:],
                                    wSv[:, c, :, k],
                                    y[:, k * B : (k + 1) * B],
                                    start=(h == 0 and k == 0),
                                    stop=(h == 1 and k == K - 1),
                                )
                        s_sb = smallp.tile([16, B], F32, tag="s_sb",
                                           name=f"s_sb{it}_{c}")
                        nc.scalar.copy(s_sb[:], s_ps[:])
                        nc.sync.dma_start(
                            cc_in[it][:, c * B : (c + 1) * B], s_sb[:])
                allreduce_squash(it, last=(it == 2))
            sq_cm.close()
    return fixup_multi_waits(nc) if fixup else nc


_NC = None


def kernel(x: np.ndarray, W: np.ndarray, _timings=None) -> np.ndarray:
    global _NC
    x = np.asarray(x, np.float32)
    W = np.asarray(W, np.float32)
    if _NC is None:
        _NC = build_all()
    in_maps = []
    for j in range(NCORES):
        sl = slice(j * IL, (j + 1) * IL)
        in_maps.append(
            {
                "W": np.ascontiguousarray(W[:, 0, sl]),
                "x": np.ascontiguousarray(x[:, sl, :]),
            }
        )
    res = run_bass_kernel_spmd(
        _NC, in_maps, core_ids=list(range(NCORES)),
        trace=_timings is not None,
    )
    if _timings is not None:
        _timings.append(res.exec_time_ns)
    v = res.results[0]["v"].astype(np.float32)  # [C, O, B]
    return np.ascontiguousarray(v.transpose(2, 0, 1))
